# revision 48
# baseline (speedup 1.0000x reference)
"""Trainium2 Bass kernel for the MLPConstructor2 adjacency problem.

Computes, per batch b (one NeuronCore each, 8-way data parallel over B):
    adj[i, j] = tanh(relu(x1_i @ w1 + x2_j @ w2 + b))
for the four (spatial/temporal) quadrants of a (2560, 2560) output,
stored as fp16 (widened to f32 on the host; quantization error ~4e-3
against a 2e-2 gate).

The output is an outer broadcast-sum of per-row and per-column scalar
vectors. The kernel is ScalarE-bound: every output element takes one
tanh slot (1 elem/cycle/lane at ~1.2 GHz, any dtype -> ~43 us floor),
with the 13.1 MB/core fp16 store (~40 us at ~330 GB/s) just under it.

Design notes (learned from traces):
- Every dma_start costs ~0.65us sequencer issue + ~0.65us DGE latency +
  ~0.9us completion-semaphore propagation, and all HWDGE rings share 4
  hardware queues, so the startup critical path is dominated by DMA
  count and queue contention, not bytes.
- The (t p)-layout x stages used previously are 128B-chunk scattered
  reads (~7us of queue time); instead ALL stats are computed from the
  fast (p t) stages and the row-side stats are permuted (p t)->(t p)
  through a DRAM scratch bounce (store contiguous, gather back with a
  4B-strided AP), like the col-side broadcast bounce.
- Col vectors are replicated across partitions with a partition-step-0
  broadcast reload of their scratch, in fp16 to halve the transfer.
- Quadrant biases are folded into the row-side stats before the bounce.
- Sem waits are engine-stream-count based, so emission order doubles as
  scheduling priority: the ss/st chains are emitted before ts/tt.
- Ring split: Sync = x stages + col bounces + ts/tt row bounces + even
  stores; Scalar = early weights/biases + ss/st row bounces (all done
  before its first TANH); GpSimd (slow SWDGE, ~3.2us per broadcast) =
  late-needed col weights + odd stores. A dummy [128,1] tanh at t=0
  preloads the activation table.
- Main loop per 128-row tile: VectorE precomputes s' = max(col + r, 0)
  with two fused dual-op tensor_scalars (fp16 4x mode, ~0.9us), ScalarE
  runs ONE bias-free tanh over the whole [128, 2560] row (2318ns --
  tanh(relu(s)) exactly), then one 655 KB contiguous store; the final
  stores are split across rings to cut the drain tail.
  (A tanh-polynomial offload to VectorE was evaluated and rejected: a
  probe showed scalar_tensor_tensor runs at 1 elem/cycle even on fp16,
  so a 6-op degree-7 chain costs ~4us/tile vs ScalarE's 2.3us.)
"""

import numpy as np
from contextlib import ExitStack

import concourse.bacc as bacc
import concourse.mybir as mybir
import concourse.tile as tile
from concourse.bass_utils import run_bass_kernel_spmd

B, N, T, D = 8, 2048, 512, 32
W = N + T            # 2560
NT, TT = N // 128, T // 128   # 16, 4 row-tiles
F32 = mybir.dt.float32
FP16 = mybir.dt.float16
QUADS = ("ss", "st", "ts", "tt")


def _emit(tc, sp, tm, ws, scr, adj):
    nc = tc.nc
    AF = mybir.ActivationFunctionType
    OP = mybir.AluOpType
    with ExitStack() as ctx:
        const = ctx.enter_context(tc.tile_pool(name="const", bufs=1))
        outp = ctx.enter_context(tc.tile_pool(name="outp", bufs=8))

        # dummy tanh: pulls ACT_TABLE_LOAD off the first real TANH
        dummy = const.tile([128, 1], F32, name="dummy")
        nc.vector.memset(dummy[:], 0.0)
        nc.scalar.activation(dummy[:], dummy[:], AF.Tanh)

        # ---- stage inputs on the Sync ring: (p t) layouts only ------------
        # (p t): row p*nt+t at [p, t*D:(t+1)*D], 2KB contiguous per line.
        x_sp_pt = const.tile([128, NT * D], F32, name="x_sp_pt")
        nc.sync.dma_start(x_sp_pt[:], sp.rearrange("(p t) d -> p t d", p=128))
        x_tm_pt = const.tile([128, TT * D], F32, name="x_tm_pt")
        nc.sync.dma_start(x_tm_pt[:], tm.rearrange("(p t) d -> p t d", p=128))

        # ---- weight/bias broadcasts, one tile per DMA ---------------------
        def wtile(name, nm, half, eng):
            t = const.tile([128, D], F32, name=name)
            src = ws[f"w_{nm}"][half * D : (half + 1) * D]
            eng.dma_start(t[:], src.unsqueeze(0).broadcast_to((128, D)))
            return t

        def btile(name, nm, eng):
            t = const.tile([128, 1], F32, name=name)
            eng.dma_start(t[:], ws[f"b_{nm}"].unsqueeze(0).broadcast_to((128, 1)))
            return t

        # Scalar ring: the ss/st-chain weights (done before its first TANH).
        w_css = wtile("w_css", "ss", 1, nc.scalar)
        w_rss = wtile("w_rss", "ss", 0, nc.scalar)
        b_ss = btile("b_ss", "ss", nc.scalar)
        w_rst = wtile("w_rst", "st", 0, nc.scalar)
        b_st = btile("b_st", "st", nc.scalar)
        b_ts = btile("b_ts", "ts", nc.sync)
        b_tt = btile("b_tt", "tt", nc.sync)
        # GpSimd ring: col-side weights needed a few us later
        w_cst = wtile("w_cst", "st", 1, nc.gpsimd)
        w_cts = wtile("w_cts", "ts", 1, nc.gpsimd)
        w_ctt = wtile("w_ctt", "tt", 1, nc.gpsimd)

        # ---- stats on VectorE (all from (p t) data) -----------------------
        col_sp_n = const.tile([128, N], FP16, name="col_sp_n")
        col_sp_t = const.tile([128, T], FP16, name="col_sp_t")
        col_tm_n = const.tile([128, N], FP16, name="col_tm_n")
        col_tm_t = const.tile([128, T], FP16, name="col_tm_t")

        def mulred(x, nt, w, name):
            prod = const.tile([128, nt * D], F32, name=f"prod_{name}")
            x3 = x[:].rearrange("p (t d) -> p t d", t=nt)
            p3 = prod[:].rearrange("p (t d) -> p t d", t=nt)
            nc.vector.tensor_tensor(
                p3, x3, w[:].unsqueeze(1).broadcast_to((128, nt, D)), OP.mult
            )
            st = const.tile([128, nt], F32, name=f"stat_{name}")
            nc.vector.tensor_reduce(st[:], p3, axis=mybir.AxisListType.X, op=OP.add)
            return st

        def cstat(x, nt, w, scr_t, col_dst, name, ring=nc.sync):
            # col-side slot: f32 mul/reduce, cast fp16, bounce through DRAM
            # scratch, partition-broadcast reload into col_dst.
            st32 = mulred(x, nt, w, f"c{name}")
            st = const.tile([128, nt], FP16, name=f"cstat_{name}")
            nc.vector.tensor_scalar_add(st[:], st32[:], 0.0)
            n = 128 * nt
            ring.dma_start(scr_t[0:n], st[:])
            ring.dma_start(col_dst, scr_t[0:n].unsqueeze(0).broadcast_to((128, n)))

        def rstat_store(x, nt, w, b, scr_t, name, ring):
            # row-side slot: stat + quadrant bias in (p t), stored to DRAM
            # scratch contiguous in row order
            st = mulred(x, nt, w, f"r{name}")
            nc.vector.tensor_scalar_add(st[:], st[:], b[:])
            ring.dma_start(scr_t.rearrange("(p t) -> p t", p=128), st[:])

        def rstat_gather(nt, scr_t, name, ring, split=1):
            # gather back in (t p) layout (f32, 4B-strided reads); split
            # across DMAs so the scattered reads run on parallel queues
            r_tp = const.tile([128, nt], F32, name=f"rtp_{name}")
            src = scr_t.rearrange("(t p) -> p t", p=128)
            q = nt // split
            for c in range(split):
                ring.dma_start(
                    r_tp[:, c * q : (c + 1) * q], src[:, c * q : (c + 1) * q]
                )
            return r_tp

        # ss/st chains first (they gate the first TANHs), ts/tt deferred
        # until after the first row-blocks so their transfers stay out of
        # the shared DMA queues during the critical window. Both row-stat
        # stores issue before either gather so the two store->gather
        # completion waits overlap on the Scalar ring.
        # css stats first: its store->broadcast has ~7us of downstream hops
        # and only 3 DVE ops of prerequisite; the r chains launch right
        # behind it and their gathers overlap the broadcast.
        cstat(x_sp_pt, NT, w_css, scr["c_ss"], col_sp_n[:], "ss")
        rstat_store(x_sp_pt, NT, w_rss, b_ss, scr["r_ss"], "ss", nc.scalar)
        rstat_store(x_sp_pt, NT, w_rst, b_st, scr["r_st"], "st", nc.scalar)
        r_ss = rstat_gather(NT, scr["r_ss"], "ss", nc.scalar, split=2)
        r_st = rstat_gather(NT, scr["r_st"], "st", nc.scalar, split=2)
        cstat(x_tm_pt, TT, w_cst, scr["c_st"], col_sp_t[:], "st")

        # ---- main loop: 20 output row-tiles of [128, 2560] ----------------
        # DVE precomputes s' = max(col + r, 0) per quadrant with one fused
        # dual-op tensor_scalar each (fp16 4x mode), so ScalarE does a
        # single bias-free tanh over the whole row tile: 2323ns vs 2505ns
        # for two biased activations, and the separate relu pass vanishes
        # (tanh(relu(s)) computed directly).
        spool = ctx.enter_context(tc.tile_pool(name="spool", bufs=4))

        def row_block(k, row0, col_n, col_t, r_n, r_t, t, last=0):
            s_ = spool.tile([128, W], FP16, name=f"s{k}", tag="s")
            nc.vector.tensor_scalar(
                s_[:, 0:N], col_n[:], r_n[:, t : t + 1], 0.0, OP.add, OP.max
            )
            nc.vector.tensor_scalar(
                s_[:, N:W], col_t[:], r_t[:, t : t + 1], 0.0, OP.add, OP.max
            )
            ot = outp.tile([128, W], FP16, name=f"ot{k}", tag="ot")
            nc.scalar.activation(ot[:], s_[:], AF.Tanh)
            if last >= 2:
                # split the final stores across rings to shrink the drain tail
                for eng, lo, hi in ((nc.sync, 0, 1024), (nc.gpsimd, 1024, 2048),
                                    (nc.scalar, 2048, W)):
                    eng.dma_start(adj[row0 : row0 + 128, lo:hi], ot[:, lo:hi])
            elif last == 1:
                for eng, lo, hi in ((nc.sync, 0, 1280), (nc.gpsimd, 1280, W)):
                    eng.dma_start(adj[row0 : row0 + 128, lo:hi], ot[:, lo:hi])
            else:
                eng = nc.sync if k % 2 == 0 else nc.gpsimd
                eng.dma_start(adj[row0 : row0 + 128, :], ot[:])

        for t in range(NT):
            row_block(t, 128 * t, col_sp_n, col_sp_t, r_ss, r_st, t)

        # ts/tt chains: emitted (= prioritized) after ALL sp row-blocks so
        # their descriptors never crowd the startup-critical Sync window;
        # results are only needed from tile 16 (~40us in) onward
        w_rts = wtile("w_rts", "ts", 0, nc.sync)
        w_rtt = wtile("w_rtt", "tt", 0, nc.sync)
        cstat(x_sp_pt, NT, w_cts, scr["c_ts"], col_tm_n[:], "ts")
        cstat(x_tm_pt, TT, w_ctt, scr["c_tt"], col_tm_t[:], "tt")
        rstat_store(x_tm_pt, TT, w_rts, b_ts, scr["r_ts"], "ts", nc.sync)
        rstat_store(x_tm_pt, TT, w_rtt, b_tt, scr["r_tt"], "tt", nc.sync)
        r_ts = rstat_gather(TT, scr["r_ts"], "ts", nc.sync)
        r_tt = rstat_gather(TT, scr["r_tt"], "tt", nc.sync)
        for t in range(TT):
            row_block(NT + t, N + 128 * t, col_tm_n, col_tm_t, r_ts, r_tt, t,
                      last=max(0, t - (TT - 3)))


def build_nc(num_devices=8):
    nc = bacc.Bacc(
        "TRN2",
        target_bir_lowering=False,
        debug=False,
        enable_asserts=False,
        num_devices=num_devices,
    )
    sp = nc.dram_tensor("spatial_nodes", (N, D), F32, kind="ExternalInput").ap()
    tm = nc.dram_tensor("temporal_nodes", (T, D), F32, kind="ExternalInput").ap()
    ws = {}
    for nm in QUADS:
        ws[f"w_{nm}"] = nc.dram_tensor(f"w_{nm}", (2 * D,), F32, kind="ExternalInput").ap()
        ws[f"b_{nm}"] = nc.dram_tensor(f"b_{nm}", (1,), F32, kind="ExternalInput").ap()
    scr = {}
    for nm, sz in (("ss", N), ("st", T), ("ts", N), ("tt", T)):
        scr[f"c_{nm}"] = nc.dram_tensor(f"scr_c_{nm}", (sz,), FP16, kind="Internal").ap()
    for nm, sz in (("ss", N), ("st", N), ("ts", T), ("tt", T)):
        scr[f"r_{nm}"] = nc.dram_tensor(f"scr_r_{nm}", (sz,), F32, kind="Internal").ap()
    adj = nc.dram_tensor("adj", (W, W), FP16, kind="ExternalOutput").ap()

    with tile.TileContext(nc) as tc:
        _emit(tc, sp, tm, ws, scr, adj)
    nc.compile()
    return nc


def make_in_maps(inputs):
    in_maps = []
    for b in range(B):
        m = {
            "spatial_nodes": np.ascontiguousarray(inputs["spatial_nodes"][b], np.float32),
            "temporal_nodes": np.ascontiguousarray(inputs["temporal_nodes"][b], np.float32),
        }
        for nm in QUADS:
            m[f"w_{nm}"] = np.ascontiguousarray(inputs[f"w_{nm}"], np.float32)
            m[f"b_{nm}"] = np.ascontiguousarray(inputs[f"b_{nm}"], np.float32)
        in_maps.append(m)
    return in_maps


_NC = {}


def run(inputs, trace=False, trace_cores=None):
    if 8 not in _NC:
        _NC[8] = build_nc(8)
    res = run_bass_kernel_spmd(
        _NC[8], make_in_maps(inputs), core_ids=list(range(B)), trace=trace,
        trace_cores=trace_cores,
    )
    out = np.stack(
        [np.asarray(res.results[i]["adj"]).astype(np.float32) for i in range(B)],
        axis=0,
    )
    return out, res


def kernel(**inputs) -> np.ndarray:
    out, _ = run(inputs, trace=False)
    return out


# revision 50
# speedup vs baseline: 1.1178x; 1.1178x over previous
"""Trainium2 Bass kernel for the MLPConstructor2 adjacency problem.

Computes, per batch b (one NeuronCore each, 8-way data parallel over B):
    adj[i, j] = tanh(relu(x1_i @ w1 + x2_j @ w2 + b))
for the four (spatial/temporal) quadrants of a (2560, 2560) output,
stored as fp16 (widened to f32 on the host; quantization error ~4e-3
against a 2e-2 gate).

The output is an outer broadcast-sum of per-row and per-column scalar
vectors. The kernel is ScalarE-bound: every output element takes one
tanh slot (1 elem/cycle/lane at ~1.2 GHz, any dtype -> ~43 us floor),
with the 13.1 MB/core fp16 store (~40 us at ~330 GB/s) just under it.

Design notes (learned from traces):
- Every dma_start costs ~0.65us sequencer issue + ~0.65us DGE latency +
  ~0.9us completion-semaphore propagation, and all HWDGE rings share 4
  hardware queues, so the startup critical path is dominated by DMA
  count and queue contention, not bytes.
- The (t p)-layout x stages used previously are 128B-chunk scattered
  reads (~7us of queue time); instead ALL stats are computed from the
  fast (p t) stages and the row-side stats are permuted (p t)->(t p)
  through a DRAM scratch bounce (store contiguous, gather back with a
  4B-strided AP), like the col-side broadcast bounce.
- Col vectors are replicated across partitions with a partition-step-0
  broadcast reload of their scratch, in fp16 to halve the transfer.
- Quadrant biases are folded into the row-side stats before the bounce.
- Sem waits are engine-stream-count based, so emission order doubles as
  scheduling priority: the ss/st chains are emitted before ts/tt.
- Ring split: Sync = x stages + col bounces + ts/tt row bounces + even
  stores; Scalar = early weights/biases + ss/st row bounces (all done
  before its first TANH); GpSimd (slow SWDGE, ~3.2us per broadcast) =
  late-needed col weights + odd stores. A dummy [128,1] tanh at t=0
  preloads the activation table.
- Main loop per 128-row tile: VectorE precomputes s' = max(col + r, 0)
  with two fused dual-op tensor_scalars (fp16 4x mode, ~0.9us), ScalarE
  runs ONE bias-free tanh over the whole [128, 2560] row (2318ns --
  tanh(relu(s)) exactly), then one 655 KB contiguous store; the final
  stores are split across rings to cut the drain tail.
  (A tanh-polynomial offload to VectorE was evaluated and rejected: a
  probe showed scalar_tensor_tensor runs at 1 elem/cycle even on fp16,
  so a 6-op degree-7 chain costs ~4us/tile vs ScalarE's 2.3us.)
"""

import numpy as np
from contextlib import ExitStack

import concourse.bacc as bacc
import concourse.mybir as mybir
import concourse.tile as tile
from concourse.bass_utils import run_bass_kernel_spmd

B, N, T, D = 8, 2048, 512, 32
W = N + T            # 2560
NT, TT = N // 128, T // 128   # 16, 4 row-tiles
F32 = mybir.dt.float32
FP16 = mybir.dt.float16
QUADS = ("ss", "st", "ts", "tt")


def _emit(tc, sp, tm, ws, scr, adj):
    nc = tc.nc
    AF = mybir.ActivationFunctionType
    OP = mybir.AluOpType
    with ExitStack() as ctx:
        const = ctx.enter_context(tc.tile_pool(name="const", bufs=1))
        outp = ctx.enter_context(tc.tile_pool(name="outp", bufs=8))

        # dummy tanh: pulls ACT_TABLE_LOAD off the first real TANH
        dummy = const.tile([128, 1], F32, name="dummy")
        nc.vector.memset(dummy[:], 0.0)
        nc.scalar.activation(dummy[:], dummy[:], AF.Tanh)

        # ---- stage inputs on the Sync ring: (p t) layouts only ------------
        # (p t): row p*nt+t at [p, t*D:(t+1)*D], 2KB contiguous per line.
        x_sp_pt = const.tile([128, NT * D], F32, name="x_sp_pt")
        nc.sync.dma_start(x_sp_pt[:], sp.rearrange("(p t) d -> p t d", p=128))
        x_tm_pt = const.tile([128, TT * D], F32, name="x_tm_pt")
        nc.sync.dma_start(x_tm_pt[:], tm.rearrange("(p t) d -> p t d", p=128))

        # ---- weight/bias broadcasts, one tile per DMA ---------------------
        def wtile(name, nm, half, eng):
            t = const.tile([128, D], F32, name=name)
            src = ws[f"w_{nm}"][half * D : (half + 1) * D]
            eng.dma_start(t[:], src.unsqueeze(0).broadcast_to((128, D)))
            return t

        def btile(name, nm, eng):
            t = const.tile([128, 1], F32, name=name)
            eng.dma_start(t[:], ws[f"b_{nm}"].unsqueeze(0).broadcast_to((128, 1)))
            return t

        # Scalar ring: the ss/st-chain weights (done before its first TANH).
        # w_rss first: empirically the fastest ordering (the row-stat chain
        # ends in a slow 4B-scatter gather, so it launches first).
        w_rss = wtile("w_rss", "ss", 0, nc.scalar)
        w_css = wtile("w_css", "ss", 1, nc.scalar)
        b_ss = btile("b_ss", "ss", nc.scalar)
        w_rst = wtile("w_rst", "st", 0, nc.scalar)
        b_st = btile("b_st", "st", nc.scalar)
        b_ts = btile("b_ts", "ts", nc.sync)
        b_tt = btile("b_tt", "tt", nc.sync)
        # GpSimd ring: col-side weights needed a few us later
        w_cst = wtile("w_cst", "st", 1, nc.gpsimd)
        w_cts = wtile("w_cts", "ts", 1, nc.gpsimd)
        w_ctt = wtile("w_ctt", "tt", 1, nc.gpsimd)

        # ---- stats on VectorE (all from (p t) data) -----------------------
        col_sp_n = const.tile([128, N], FP16, name="col_sp_n")
        col_sp_t = const.tile([128, T], FP16, name="col_sp_t")
        col_tm_n = const.tile([128, N], FP16, name="col_tm_n")
        col_tm_t = const.tile([128, T], FP16, name="col_tm_t")

        def mulred(x, nt, w, name):
            prod = const.tile([128, nt * D], F32, name=f"prod_{name}")
            x3 = x[:].rearrange("p (t d) -> p t d", t=nt)
            p3 = prod[:].rearrange("p (t d) -> p t d", t=nt)
            nc.vector.tensor_tensor(
                p3, x3, w[:].unsqueeze(1).broadcast_to((128, nt, D)), OP.mult
            )
            st = const.tile([128, nt], F32, name=f"stat_{name}")
            nc.vector.tensor_reduce(st[:], p3, axis=mybir.AxisListType.X, op=OP.add)
            return st

        def cstat(x, nt, w, scr_t, col_dst, name, ring=nc.sync):
            # col-side slot: f32 mul/reduce, cast fp16, bounce through DRAM
            # scratch, partition-broadcast reload into col_dst.
            st32 = mulred(x, nt, w, f"c{name}")
            st = const.tile([128, nt], FP16, name=f"cstat_{name}")
            nc.vector.tensor_scalar_add(st[:], st32[:], 0.0)
            n = 128 * nt
            ring.dma_start(scr_t[0:n], st[:])
            ring.dma_start(col_dst, scr_t[0:n].unsqueeze(0).broadcast_to((128, n)))

        def rstat_store(x, nt, w, b, scr_t, name, ring):
            # row-side slot: stat + quadrant bias in (p t), stored to DRAM
            # scratch contiguous in row order
            st = mulred(x, nt, w, f"r{name}")
            nc.vector.tensor_scalar_add(st[:], st[:], b[:])
            ring.dma_start(scr_t.rearrange("(p t) -> p t", p=128), st[:])

        def rstat_gather(nt, scr_t, name, ring, split=1):
            # gather back in (t p) layout (f32, 4B-strided reads); split
            # across DMAs so the scattered reads run on parallel queues
            r_tp = const.tile([128, nt], F32, name=f"rtp_{name}")
            src = scr_t.rearrange("(t p) -> p t", p=128)
            q = nt // split
            for c in range(split):
                ring.dma_start(
                    r_tp[:, c * q : (c + 1) * q], src[:, c * q : (c + 1) * q]
                )
            return r_tp

        # ss/st chains first (they gate the first TANHs), ts/tt deferred
        # until after the first row-blocks so their transfers stay out of
        # the shared DMA queues during the critical window. Both row-stat
        # stores issue before either gather so the two store->gather
        # completion waits overlap on the Scalar ring.
        rstat_store(x_sp_pt, NT, w_rss, b_ss, scr["r_ss"], "ss", nc.scalar)
        rstat_store(x_sp_pt, NT, w_rst, b_st, scr["r_st"], "st", nc.scalar)
        cstat(x_sp_pt, NT, w_css, scr["c_ss"], col_sp_n[:], "ss")
        r_ss = rstat_gather(NT, scr["r_ss"], "ss", nc.scalar, split=2)
        r_st = rstat_gather(NT, scr["r_st"], "st", nc.scalar, split=2)
        cstat(x_tm_pt, TT, w_cst, scr["c_st"], col_sp_t[:], "st")

        # ---- main loop: 20 output row-tiles of [128, 2560] ----------------
        # DVE precomputes s' = max(col + r, 0) per quadrant with one fused
        # dual-op tensor_scalar each (fp16 4x mode), so ScalarE does a
        # single bias-free tanh over the whole row tile: 2323ns vs 2505ns
        # for two biased activations, and the separate relu pass vanishes
        # (tanh(relu(s)) computed directly).
        spool = ctx.enter_context(tc.tile_pool(name="spool", bufs=4))

        def row_block(k, row0, col_n, col_t, r_n, r_t, t, last=0):
            s_ = spool.tile([128, W], FP16, name=f"s{k}", tag="s")
            nc.vector.tensor_scalar(
                s_[:, 0:N], col_n[:], r_n[:, t : t + 1], 0.0, OP.add, OP.max
            )
            nc.vector.tensor_scalar(
                s_[:, N:W], col_t[:], r_t[:, t : t + 1], 0.0, OP.add, OP.max
            )
            ot = outp.tile([128, W], FP16, name=f"ot{k}", tag="ot")
            nc.scalar.activation(ot[:], s_[:], AF.Tanh)
            if last >= 2:
                # split the final stores across rings to shrink the drain tail
                for eng, lo, hi in ((nc.sync, 0, 1024), (nc.gpsimd, 1024, 2048),
                                    (nc.scalar, 2048, W)):
                    eng.dma_start(adj[row0 : row0 + 128, lo:hi], ot[:, lo:hi])
            elif last == 1:
                for eng, lo, hi in ((nc.sync, 0, 1280), (nc.gpsimd, 1280, W)):
                    eng.dma_start(adj[row0 : row0 + 128, lo:hi], ot[:, lo:hi])
            else:
                eng = nc.sync if k % 2 == 0 else nc.gpsimd
                eng.dma_start(adj[row0 : row0 + 128, :], ot[:])

        for t in range(NT):
            row_block(t, 128 * t, col_sp_n, col_sp_t, r_ss, r_st, t)

        # ts/tt chains: emitted (= prioritized) after ALL sp row-blocks so
        # their descriptors never crowd the startup-critical Sync window;
        # results are only needed from tile 16 (~40us in) onward
        w_rts = wtile("w_rts", "ts", 0, nc.sync)
        w_rtt = wtile("w_rtt", "tt", 0, nc.sync)
        cstat(x_sp_pt, NT, w_cts, scr["c_ts"], col_tm_n[:], "ts")
        cstat(x_tm_pt, TT, w_ctt, scr["c_tt"], col_tm_t[:], "tt")
        rstat_store(x_tm_pt, TT, w_rts, b_ts, scr["r_ts"], "ts", nc.sync)
        rstat_store(x_tm_pt, TT, w_rtt, b_tt, scr["r_tt"], "tt", nc.sync)
        r_ts = rstat_gather(TT, scr["r_ts"], "ts", nc.sync)
        r_tt = rstat_gather(TT, scr["r_tt"], "tt", nc.sync)
        for t in range(TT):
            row_block(NT + t, N + 128 * t, col_tm_n, col_tm_t, r_ts, r_tt, t,
                      last=max(0, t - (TT - 3)))


def build_nc(num_devices=8):
    nc = bacc.Bacc(
        "TRN2",
        target_bir_lowering=False,
        debug=False,
        enable_asserts=False,
        num_devices=num_devices,
    )
    sp = nc.dram_tensor("spatial_nodes", (N, D), F32, kind="ExternalInput").ap()
    tm = nc.dram_tensor("temporal_nodes", (T, D), F32, kind="ExternalInput").ap()
    ws = {}
    for nm in QUADS:
        ws[f"w_{nm}"] = nc.dram_tensor(f"w_{nm}", (2 * D,), F32, kind="ExternalInput").ap()
        ws[f"b_{nm}"] = nc.dram_tensor(f"b_{nm}", (1,), F32, kind="ExternalInput").ap()
    scr = {}
    for nm, sz in (("ss", N), ("st", T), ("ts", N), ("tt", T)):
        scr[f"c_{nm}"] = nc.dram_tensor(f"scr_c_{nm}", (sz,), FP16, kind="Internal").ap()
    for nm, sz in (("ss", N), ("st", N), ("ts", T), ("tt", T)):
        scr[f"r_{nm}"] = nc.dram_tensor(f"scr_r_{nm}", (sz,), F32, kind="Internal").ap()
    adj = nc.dram_tensor("adj", (W, W), FP16, kind="ExternalOutput").ap()

    with tile.TileContext(nc) as tc:
        _emit(tc, sp, tm, ws, scr, adj)
    nc.compile()
    return nc


def make_in_maps(inputs):
    in_maps = []
    for b in range(B):
        m = {
            "spatial_nodes": np.ascontiguousarray(inputs["spatial_nodes"][b], np.float32),
            "temporal_nodes": np.ascontiguousarray(inputs["temporal_nodes"][b], np.float32),
        }
        for nm in QUADS:
            m[f"w_{nm}"] = np.ascontiguousarray(inputs[f"w_{nm}"], np.float32)
            m[f"b_{nm}"] = np.ascontiguousarray(inputs[f"b_{nm}"], np.float32)
        in_maps.append(m)
    return in_maps


_NC = {}


def run(inputs, trace=False, trace_cores=None):
    if 8 not in _NC:
        _NC[8] = build_nc(8)
    res = run_bass_kernel_spmd(
        _NC[8], make_in_maps(inputs), core_ids=list(range(B)), trace=trace,
        trace_cores=trace_cores,
    )
    out = np.stack(
        [np.asarray(res.results[i]["adj"]).astype(np.float32) for i in range(B)],
        axis=0,
    )
    return out, res


def kernel(**inputs) -> np.ndarray:
    out, _ = run(inputs, trace=False)
    return out
